# revision 19
# baseline (speedup 1.0000x reference)
"""Trainium2 Bass kernel for nn_Classifier_36618891166176 (R-GCN message passing).

Strategy (8 NeuronCores, SPMD single program):
  - Partition dst nodes across cores (12500 each). Each core processes the
    edges whose dst lies in its range, for all 4 relations and all 3 layers.
  - Per layer: gather h[src] rows via gpsimd dma_gather (4 SWDGE queues
    round-robin), aggregate per 128-dst window with one-hot Sel matmuls into
    PSUM (normalization coefficients folded into Sel), apply the per-relation
    weight with a second matmul chain, relu, write the core's h slice, then
    AllGather slices into the full h for the next layer.
  - Sel one-hot matrices are layer-invariant; they are precomputed on host
    (ce folded in) and STREAMED from DRAM per dst-window instead of being
    generated on the vector engine (which was the baseline bottleneck).
  - AvgPool per graph id via the same one-hot-matmul trick with
    host-precomputed pool-sel slabs, partial [128 feat, 256 graphs] per core,
    AllReduce, then the classifier matmul on every core.

dma_gather indices are int16 (max 32768 rows), so the gather source h is
addressed in 4 chunks of 25000 rows; edges are laid out in
(window, chunk, relation) cell order, each cell padded to a multiple of 128
(one PE tile), cell sizes equalized across cores so one program serves all 8.
"""
import os
import sys
import numpy as np

import concourse.bacc as bacc
import concourse.bass as bass
import concourse.mybir as mybir
import concourse.tile as tile
from concourse.bass_utils import run_bass_kernel_spmd
from concourse import library_config

# Problem constants (hardcoded per harness contract).
N = 100000
E = 1000000
R = 4
G = 256
D = 128          # feature dim (DIN == DH == 128)
NCLS = 10
L = 3
NCORES = 8
NSLICE = N // NCORES          # 12500 dst nodes per core
NW = (NSLICE + 127) // 128    # 98 dst windows per core
LAST_W_ROWS = NSLICE - (NW - 1) * 128  # 84
CHUNK = 25000
NCHUNK = (N + CHUNK - 1) // CHUNK      # 4
P = 128

_CACHE = {}


def _prep(edges, graph_ids):
    """Host-side edge preprocessing. Layer-invariant. Returns per-core arrays
    plus the uniform schedule."""
    import ml_dtypes
    bf = ml_dtypes.bfloat16
    edges = np.asarray(edges)
    graph_ids = np.asarray(graph_ids)

    # Degrees + per-edge normalization coefficient (global, per relation).
    ce_all = []
    for r in range(R):
        src = edges[r, 0].astype(np.int64)
        dst = edges[r, 1].astype(np.int64)
        deg_out = np.maximum(np.bincount(src, minlength=N), 1.0)
        deg_in = np.maximum(np.bincount(dst, minlength=N), 1.0)
        ce_all.append((1.0 / np.sqrt(deg_out[src]) / np.sqrt(deg_in[dst]))
                      .astype(np.float32))

    # Per-core cell partition: key = (w, chunk, r).
    # cells[c][(w,ch,r)] = (src_local int32, dst_slot int32, ce f32)
    cells = [dict() for _ in range(NCORES)]
    for r in range(R):
        src = edges[r, 0].astype(np.int64)
        dst = edges[r, 1].astype(np.int64)
        ce = ce_all[r]
        core = dst // NSLICE
        local = dst - core * NSLICE
        w = local >> 7
        slot = local & 127
        ch = src // CHUNK
        srcl = src % CHUNK
        key_sort = (((core * NW + w) * NCHUNK + ch))
        order = np.argsort(key_sort, kind="stable")
        so, wo, cho, slo, srco, ceo = (core[order], w[order], ch[order],
                                       slot[order], srcl[order], ce[order])
        keys = ((so * NW + wo) * NCHUNK + cho)
        bounds = np.searchsorted(keys, np.arange(NCORES * NW * NCHUNK + 1))
        for k in range(NCORES * NW * NCHUNK):
            a, b = bounds[k], bounds[k + 1]
            if a == b:
                continue
            c = k // (NW * NCHUNK)
            rem = k % (NW * NCHUNK)
            wk, chk = rem // NCHUNK, rem % NCHUNK
            cells[c][(wk, chk, r)] = (srco[a:b], slo[a:b], ceo[a:b])

    # Uniform tile counts per cell (max across cores).
    ntiles = np.zeros((NW, NCHUNK, R), dtype=np.int64)
    for w in range(NW):
        for ch in range(NCHUNK):
            for r in range(R):
                mx = 0
                for c in range(NCORES):
                    t = cells[c].get((w, ch, r))
                    if t is not None:
                        mx = max(mx, len(t[0]))
                ntiles[w, ch, r] = (mx + 127) // 128
    TT = int(ntiles.sum())           # total tiles per core per layer
    EPAD = TT * P

    # Flat per-core edge arrays in global tile order (w, ch, r, tile).
    src16 = np.zeros((NCORES, EPAD), dtype=np.int16)
    dsts = np.zeros((NCORES, EPAD), dtype=np.int64)
    cef = np.zeros((NCORES, EPAD), dtype=np.float32)
    pos = 0
    cell_tile_base = np.zeros((NW, NCHUNK, R), dtype=np.int64)
    for w in range(NW):
        for ch in range(NCHUNK):
            for r in range(R):
                nt = int(ntiles[w, ch, r])
                cell_tile_base[w, ch, r] = pos // P
                if nt == 0:
                    continue
                for c in range(NCORES):
                    t = cells[c].get((w, ch, r))
                    if t is None:
                        continue
                    n = len(t[0])
                    src16[c, pos:pos + n] = t[0].astype(np.int16)
                    dsts[c, pos:pos + n] = t[1]
                    cef[c, pos:pos + n] = t[2]
                pos += nt * P
    assert pos == EPAD
    del cells

    # Gather idx tensor: [128, EPAD/16] int16, batch-local wrap.
    # Batches are per (w, ch): positions [base, base+n) with n = 128*sum_r nt.
    gidx = np.zeros((NCORES, P, EPAD // 16), dtype=np.int16)
    batches = []   # (w, ch, tile_base, n_tiles)
    for w in range(NW):
        for ch in range(NCHUNK):
            nt = int(ntiles[w, ch].sum())
            if nt == 0:
                continue
            tb = int(cell_tile_base[w, ch, 0])
            batches.append((w, ch, tb, nt))
            n = nt * P
            base = tb * P
            k = np.arange(n)
            col0 = base // 16
            for c in range(NCORES):
                vals = src16[c, base:base + n]
                lay = np.zeros((16, n // 16), dtype=np.int16)
                lay[k % 16, k // 16] = vals
                gidx[c, :, col0:col0 + n // 16] = np.tile(lay, (8, 1))

    # Host-precomputed sel one-hot slabs: [128, TT, 128] fp8(e4m3) per core.
    # sel[p, t, d] = ce(edge t*128+p) if d == dst_slot(edge) else 0.
    f8 = ml_dtypes.float8_e4m3
    selh = np.zeros((NCORES, P, TT, P), dtype=f8)
    kk = np.arange(EPAD)
    pp = (kk % P)
    tt_ = (kk // P)
    for c in range(NCORES):
        selh[c, pp, tt_, dsts[c]] = cef[c].astype(f8)

    # Pool sel slabs: [128, NW, 256] bf16 per core.
    # psel[p, w, g] = 1/count[g] if graph_ids[c*NSLICE + w*128 + p] == g.
    counts = np.maximum(np.bincount(graph_ids.astype(np.int64), minlength=G),
                        1.0).astype(np.float32)
    psel = np.zeros((NCORES, P, NW, G), dtype=bf)
    for c in range(NCORES):
        g = graph_ids[c * NSLICE:(c + 1) * NSLICE].astype(np.int64)
        nodes = np.arange(NSLICE)
        wv = nodes >> 7
        pv = nodes & 127
        psel[c, pv, wv, g] = (1.0 / counts[g]).astype(bf)

    sched = {
        "ntiles": ntiles, "cell_tile_base": cell_tile_base,
        "TT": TT, "EPAD": EPAD, "batches": batches,
    }
    arrays = {
        "gidx": gidx, "selh": selh, "psel": psel,
    }
    return sched, arrays


def _build(sched):
    """Build the SPMD bass program for the uniform schedule."""
    ntiles = sched["ntiles"]
    cell_tile_base = sched["cell_tile_base"]
    TT = sched["TT"]
    EPAD = sched["EPAD"]
    batches = sched["batches"]
    max_batch_tiles = max(b[3] for b in batches)
    # max tiles in one whole dst window (for the sel slab tile size)
    wnt_list = []
    for w in range(NW):
        wt0 = int(cell_tile_base[w, 0, 0])
        wt1 = int(cell_tile_base[w + 1, 0, 0]) if w + 1 < NW else TT
        wnt_list.append(wt1 - wt0)
    max_w_tiles = max(wnt_list)

    f32 = mybir.dt.float32
    bf16 = mybir.dt.bfloat16
    i16 = mybir.dt.int16

    nc = bacc.Bacc("TRN2", target_bir_lowering=False, debug=False,
                   num_swdge_queues=4, dynamic_dma_scratch_size=65536)

    feat = nc.dram_tensor("feat", [N, D], bf16, kind="ExternalInput")
    wstack = nc.dram_tensor("wstack", [L * R, D, D], bf16, kind="ExternalInput")
    wc = nc.dram_tensor("wc", [D, NCLS], f32, kind="ExternalInput")
    bcb = nc.dram_tensor("bcb", [NCLS, 1], f32, kind="ExternalInput")
    gidx_d = nc.dram_tensor("gidx", [P, EPAD // 16], i16, kind="ExternalInput")
    sel_d = nc.dram_tensor("sel", [P, TT * P], mybir.dt.float8e4, kind="ExternalInput")
    psel_d = nc.dram_tensor("psel", [P, NW * G], bf16, kind="ExternalInput")

    h_full = [None,
              nc.dram_tensor("h1f", [N, D], bf16, addr_space="Shared"),
              nc.dram_tensor("h2f", [N, D], bf16, addr_space="Shared")]
    h_slice = [nc.dram_tensor(f"hs{l}", [NSLICE, D], bf16) for l in range(L)]
    poolin = nc.dram_tensor("poolin", [P, G], f32)
    poolout = nc.dram_tensor("poolout", [P, G], f32, addr_space="Shared")
    out_d = nc.dram_tensor("out", [NCLS, G], f32, kind="ExternalOutput")

    cc_sem = nc.alloc_semaphore("ccsem")
    cc_count = [0]

    with tile.TileContext(nc) as tc:
        nc.gpsimd.load_library(library_config.mlp)

        with tc.tile_pool(name="const", bufs=1) as cpool:
            w_sb = []
            for i in range(L * R):
                t = cpool.tile([D, D], bf16, tag=f"w{i}")
                nc.sync.dma_start(t[:], wstack[i])
                w_sb.append(t)
            wc_sb = cpool.tile([D, NCLS], f32, tag="wc")
            nc.sync.dma_start(wc_sb[:], wc[:])
            bc_sb = cpool.tile([NCLS, 1], f32, tag="bc")
            nc.sync.dma_start(bc_sb[:], bcb[:])

            qrr = [0]
            nidx_regs = {}
            for b in batches:
                v = b[3] * P
                if v not in nidx_regs:
                    nidx_regs[v] = nc.gpsimd.to_reg(v)

            def run_layer(layer, table, out_slice):
                with tc.tile_pool(name=f"idx{layer}", bufs=8) as idxp, \
                     tc.tile_pool(name=f"x{layer}", bufs=24) as xp, \
                     tc.tile_pool(name=f"sel{layer}", bufs=5) as selp, \
                     tc.tile_pool(name=f"mt{layer}", bufs=6) as mtp, \
                     tc.tile_pool(name=f"ho{layer}", bufs=4) as hop, \
                     tc.tile_pool(name=f"pa{layer}", bufs=6, space="PSUM") as pap, \
                     tc.tile_pool(name=f"pb{layer}", bufs=2, space="PSUM") as pbp:
                    for w in range(NW):
                        wt0 = int(cell_tile_base[w, 0, 0])
                        wnt = wnt_list[w]
                        # stream this window's sel slab from DRAM (ACT HWDGE)
                        sel_sb = selp.tile([P, max_w_tiles, P], mybir.dt.float8e4, tag="sel")
                        nc.scalar.dma_start(
                            sel_sb[:, :wnt, :],
                            sel_d[:, wt0 * P:(wt0 + wnt) * P])

                        # one idx load for the whole window (SP HWDGE)
                        it = idxp.tile([P, max_w_tiles * 8], i16, tag="idx")
                        nc.sync.dma_start(
                            it[:, :wnt * 8],
                            gidx_d[:, wt0 * 8:(wt0 + wnt) * 8])

                        # gathers: one batch per chunk
                        xw = {}
                        for ch in range(NCHUNK):
                            nt = int(ntiles[w, ch].sum())
                            if nt == 0:
                                continue
                            tb = int(cell_tile_base[w, ch, 0])
                            nidx = nt * P
                            cols = nidx // 16
                            co = (tb - wt0) * 8
                            x = xp.tile([P, max_batch_tiles, D], bf16, tag="x")
                            rows = min(CHUNK, N - ch * CHUNK)
                            nc.gpsimd.dma_gather(
                                x[:, :nt, :], table[ch * CHUNK:ch * CHUNK + rows],
                                it[:, co:co + cols], nidx, nidx_regs[nidx], D,
                                single_packet=False, queue_num=qrr[0])
                            qrr[0] = (qrr[0] + 1) % 4
                            xw[ch] = (x, tb)

                        # aggregation matmuls
                        psa = {}
                        seen = {r: 0 for r in range(R)}
                        tot = {r: int(ntiles[w, :, r].sum()) for r in range(R)}
                        for ch in range(NCHUNK):
                            if ch not in xw:
                                continue
                            x, tb = xw[ch]
                            for r in range(R):
                                nt = int(ntiles[w, ch, r])
                                if nt == 0:
                                    continue
                                ctb = int(cell_tile_base[w, ch, r])
                                for i in range(nt):
                                    tg = ctb + i        # global tile id
                                    xi = tg - tb        # tile within batch
                                    si = tg - wt0       # tile within window
                                    if r not in psa:
                                        psa[r] = pap.tile(
                                            [P, P], f32, tag="pa",
                                            name=f"pa{layer}_{w}_{r}")
                                    nc.tensor.matmul(
                                        psa[r][:], lhsT=x[:, xi, :],
                                        rhs=sel_sb[:, si, :],
                                        start=(seen[r] == 0),
                                        stop=(seen[r] == tot[r] - 1))
                                    seen[r] += 1

                        # per-relation weight matmul + relu + store
                        psb = pbp.tile([P, P], f32, tag="pb")
                        live = [r for r in range(R) if tot[r] > 0]
                        for j, r in enumerate(live):
                            mt = mtp.tile([P, P], bf16, tag="mt")
                            nc.scalar.copy(mt[:], psa[r][:])
                            nc.tensor.matmul(
                                psb[:], lhsT=mt[:],
                                rhs=w_sb[layer * R + r][:],
                                start=(j == 0), stop=(j == len(live) - 1))
                        rows = P if w < NW - 1 else LAST_W_ROWS
                        ho = hop.tile([P, D], bf16, tag="ho")
                        nc.scalar.activation(
                            ho[:], psb[:],
                            mybir.ActivationFunctionType.Relu)
                        nc.scalar.dma_start(
                            out_slice[w * P:w * P + rows], ho[:rows, :])

            run_layer(0, feat, h_slice[0])
            tc.strict_bb_all_engine_barrier()
            with tc.tile_critical():
                cc_count[0] += 1
                nc.gpsimd.collective_compute(
                    "AllGather", mybir.AluOpType.bypass,
                    ins=[h_slice[0][:]], outs=[h_full[1][:]],
                    replica_groups=[list(range(NCORES))],
                ).then_inc(cc_sem, 1)
                nc.gpsimd.wait_ge(cc_sem, cc_count[0])
            tc.strict_bb_all_engine_barrier()

            run_layer(1, h_full[1], h_slice[1])
            tc.strict_bb_all_engine_barrier()
            with tc.tile_critical():
                cc_count[0] += 1
                nc.gpsimd.collective_compute(
                    "AllGather", mybir.AluOpType.bypass,
                    ins=[h_slice[1][:]], outs=[h_full[2][:]],
                    replica_groups=[list(range(NCORES))],
                ).then_inc(cc_sem, 1)
                nc.gpsimd.wait_ge(cc_sem, cc_count[0])
            tc.strict_bb_all_engine_barrier()

            run_layer(2, h_full[2], h_slice[2])
            tc.strict_bb_all_engine_barrier()

            # ---- pooling + classifier ----
            with tc.tile_pool(name="poolp", bufs=6) as pp, \
                 tc.tile_pool(name="pps", bufs=2, space="PSUM") as pps:
                psc = pps.tile([P, G], f32, tag="psc")
                for w in range(NW):
                    rows = P if w < NW - 1 else LAST_W_ROWS
                    ht = pp.tile([P, D], bf16, tag="ht")
                    nc.sync.dma_start(ht[:rows, :],
                                      h_slice[2][w * P:w * P + rows])
                    sg = pp.tile([P, G], bf16, tag="sg")
                    nc.sync.dma_start(sg[:], psel_d[:, w * G:(w + 1) * G])
                    nc.tensor.matmul(psc[:], lhsT=ht[:rows, :],
                                     rhs=sg[:rows, :],
                                     start=(w == 0), stop=(w == NW - 1))
                pool_sb = pp.tile([P, G], f32, tag="poolsb")
                nc.scalar.copy(pool_sb[:], psc[:])
                nc.sync.dma_start(poolin[:], pool_sb[:])
                tc.strict_bb_all_engine_barrier()
                with tc.tile_critical():
                    cc_count[0] += 1
                    nc.gpsimd.collective_compute(
                        "AllReduce", mybir.AluOpType.add,
                        ins=[poolin[:]], outs=[poolout[:]],
                        replica_groups=[list(range(NCORES))],
                    ).then_inc(cc_sem, 1)
                    nc.gpsimd.wait_ge(cc_sem, cc_count[0])
                tc.strict_bb_all_engine_barrier()
                pout = pp.tile([P, G], f32, tag="pout")
                nc.sync.dma_start(pout[:], poolout[:])
                pcls = pps.tile([NCLS, G], f32, tag="pcls")
                nc.tensor.matmul(pcls[:], lhsT=wc_sb[:], rhs=pout[:],
                                 start=True, stop=True)
                osb = pp.tile([NCLS, G], f32, tag="osb")
                nc.scalar.activation(
                    osb[:], pcls[:],
                    mybir.ActivationFunctionType.Identity, bias=bc_sb[:])
                nc.sync.dma_start(out_d[:], osb[:])

    nc.compile()
    return nc


def _get_compiled(inputs):
    key = "k"
    if key in _CACHE:
        return _CACHE[key]
    sched, arrays = _prep(inputs["edges"], inputs["graph_ids"])
    nc = _build(sched)
    _CACHE[key] = (nc, sched, arrays)
    return _CACHE[key]


def _in_maps(inputs, arrays):
    import ml_dtypes
    bf = ml_dtypes.bfloat16
    feat = np.ascontiguousarray(
        np.asarray(inputs["features"], dtype=np.float32).astype(bf))
    W0 = np.asarray(inputs["W0"], dtype=np.float32)
    Wl = np.asarray(inputs["Wl"], dtype=np.float32)
    wstack = np.concatenate([W0.reshape(R, D, D),
                             Wl.reshape((L - 1) * R, D, D)], axis=0).astype(bf)
    b0 = np.asarray(inputs["b0"])
    bl = np.asarray(inputs["bl"])
    assert np.all(b0 == 0) and np.all(bl == 0), \
        "nonzero per-relation biases not folded in this kernel"
    wc = np.asarray(inputs["Wc"], dtype=np.float32)
    bcb = np.asarray(inputs["bc"], dtype=np.float32).reshape(NCLS, 1)
    TT = arrays["selh"].shape[2]
    maps = []
    for c in range(NCORES):
        maps.append({
            "feat": feat, "wstack": wstack, "wc": wc, "bcb": bcb,
            "gidx": arrays["gidx"][c],
            "sel": arrays["selh"][c].reshape(P, TT * P),
            "psel": arrays["psel"][c].reshape(P, NW * G),
        })
    return maps


def kernel(**inputs) -> np.ndarray:
    nc, sched, arrays = _get_compiled(inputs)
    maps = _in_maps(inputs, arrays)
    res = run_bass_kernel_spmd(nc, maps, list(range(NCORES)), trace=False)
    return np.ascontiguousarray(res.results[0]["out"].T)


def kernel_traced(**inputs):
    """Like kernel() but returns (output, exec_time_ns). Used by test.py."""
    import types
    import concourse.bass_utils as bum
    if "antenv.axon_hooks" not in sys.modules:
        mod = types.ModuleType("antenv.axon_hooks")
        mod._hook = None
        mod.set_axon_ntff_profile_hook = lambda h: setattr(mod, "_hook", h)
        mod.get_axon_ntff_profile_hook = lambda: mod._hook
        sys.modules["antenv.axon_hooks"] = mod
        import antenv
        antenv.axon_hooks = mod
        from trn_agent_boot.trn_boot import _ntff_profile_via_ctypes
        mod._hook = _ntff_profile_via_ctypes('/opt/axon/libaxon_pjrt.so')
    bum.upload_artifacts = lambda tmpdir: "local://skipped"
    nc, sched, arrays = _get_compiled(inputs)
    maps = _in_maps(inputs, arrays)
    res = run_bass_kernel_spmd(nc, maps, list(range(NCORES)), trace=True)
    return np.ascontiguousarray(res.results[0]["out"].T), res.exec_time_ns
